# revision 6
# baseline (speedup 1.0000x reference)
"""Bass/Tile TRN2 kernel for nn_BertAttention (B=2, S=4096, H=768) on 8 NeuronCores.

Sharding: core c handles batch b = c // 4, query chunk qc = c % 4 (1024 queries).
Each core computes K/V projections for its full batch (4x redundant), attention
for its own 1024 queries, then Wo1 + LN1 + Wo2 + LN2 token-parallel.

All matmuls run in bf16 with fp32 PSUM accumulation; softmax and layernorms in
fp32. Scores are computed transposed (sT[k, q]) so the attention mask and the
1/sqrt(H) scale fold into the exp activation's per-partition scale operand, and
the softmax denominator comes from a ones-column appended to V.
"""

import sys

if "/opt/trn_rl_repo" not in sys.path:
    sys.path.insert(0, "/opt/trn_rl_repo")

import numpy as np
import ml_dtypes

import concourse.bass as bass
import concourse.mybir as mybir
import concourse.tile as tile
from concourse import bacc
from concourse.masks import make_identity

BF16 = mybir.dt.bfloat16
F32 = mybir.dt.float32

B, S, H = 2, 4096, 768
NQ = S // 4          # queries per core
HC = H // 128        # 6 hidden chunks
KC = S // 128        # 32 key chunks
QB = 256             # query block for attention phase
EPS = 1e-12
NCORES = 8


def _emit(nc, tc, io):
    (xT, xqT, wqT, wkT, wvT, wo1T, wo2T, bq, bk, bv, g1, be1, g2, be2,
     mscale, xb1, xb2, out) = io

    from contextlib import ExitStack
    ctx = ExitStack()
    consts = ctx.enter_context(tc.tile_pool(name="consts", bufs=1))
    wpool = ctx.enter_context(tc.tile_pool(name="wpool", bufs=3))
    kvq = ctx.enter_context(tc.tile_pool(name="kvq", bufs=1))
    xtp = ctx.enter_context(tc.tile_pool(name="xtp", bufs=3))
    ppool = ctx.enter_context(tc.tile_pool(name="ppool", bufs=3))
    ctxp = ctx.enter_context(tc.tile_pool(name="ctxp", bufs=2))
    vstr = ctx.enter_context(tc.tile_pool(name="vstr", bufs=4))
    resp = ctx.enter_context(tc.tile_pool(name="resp", bufs=3))
    h1p = ctx.enter_context(tc.tile_pool(name="h1p", bufs=2))
    smallp = ctx.enter_context(tc.tile_pool(name="smallp", bufs=8))
    outp = ctx.enter_context(tc.tile_pool(name="outp", bufs=3))
    psum = ctx.enter_context(tc.tile_pool(name="psum", bufs=2, space="PSUM"))
    vdram = ctx.enter_context(tc.tile_pool(name="vdram", bufs=KC, space="DRAM"))

    # ---- constants ----
    ident = consts.tile([128, 128], BF16, tag="ident")
    make_identity(nc, ident)

    wq_sb = wpool.tile([128, HC, H], BF16, tag="w")
    wk_sb = wpool.tile([128, HC, H], BF16, tag="w")
    wv_sb = wpool.tile([128, HC, H], BF16, tag="w")
    nc.sync.dma_start(out=wq_sb, in_=wqT.ap().rearrange("(c p) o -> p c o", p=128))
    nc.sync.dma_start(out=wk_sb, in_=wkT.ap().rearrange("(c p) o -> p c o", p=128))
    nc.sync.dma_start(out=wv_sb, in_=wvT.ap().rearrange("(c p) o -> p c o", p=128))

    bq_sb = consts.tile([128, HC], F32, tag="bq")
    bk_sb = consts.tile([128, HC], F32, tag="bk")
    nc.sync.dma_start(out=bq_sb, in_=bq.ap().rearrange("(c p) -> p c", p=128))
    nc.sync.dma_start(out=bk_sb, in_=bk.ap().rearrange("(c p) -> p c", p=128))

    def bcast(vec, tg):
        t = consts.tile([128, H], F32, tag=tg)
        v = vec.ap()
        nc.gpsimd.dma_start(
            out=t, in_=bass.AP(tensor=v.tensor, offset=v.offset, ap=[[0, 128]] + list(v.ap)))
        return t

    bv_b = bcast(bv, "bvb")
    g1_b = bcast(g1, "g1b")
    be1_b = bcast(be1, "be1b")
    g2_b = bcast(g2, "g2b")
    be2_b = bcast(be2, "be2b")

    msc_sb = consts.tile([128, KC], F32, tag="msc")
    nc.sync.dma_start(out=msc_sb, in_=mscale.ap().rearrange("(c p) -> p c", p=128))

    eps_sb = consts.tile([128, 1], F32, tag="eps")
    nc.vector.memset(eps_sb, EPS)

    # ---- resident K_H [o, k] and Q_H [o, q] (bf16) ----
    k_h = kvq.tile([128, HC, S], BF16, tag="k_h")
    q_h = kvq.tile([128, HC, NQ], BF16, tag="q_h")

    # ---- phase B: projections ----
    v_tiles = []
    for kb in range(S // 512):
        xt = xtp.tile([128, HC, 512], BF16, tag="xt")
        nc.sync.dma_start(
            out=xt, in_=xT.ap().rearrange("(c p) k -> p c k", p=128)[:, :, kb * 512:(kb + 1) * 512])
        # K projection: out [o128, k512] accumulated over h chunks
        for oc in range(HC):
            kps = psum.tile([128, 512], F32, tag="acc512")
            for hc in range(HC):
                nc.tensor.matmul(kps, wk_sb[:, hc, oc * 128:(oc + 1) * 128],
                                 xt[:, hc, :], start=(hc == 0), stop=(hc == HC - 1))
            nc.vector.tensor_scalar_add(
                out=k_h[:, oc, kb * 512:(kb + 1) * 512], in0=kps,
                scalar1=bk_sb[:, oc:oc + 1])
        # V projection: out [k128, o] tiles, spilled to DRAM (with ones col)
        for ks in range(4):
            kc = kb * 4 + ks
            vps1 = psum.tile([128, 512], F32, tag="acc512")
            vps2 = psum.tile([128, 257], F32, tag="acc257")
            for hc in range(HC):
                lhs = xt[:, hc, ks * 128:(ks + 1) * 128]
                nc.tensor.matmul(vps1, lhs, wv_sb[:, hc, 0:512],
                                 start=(hc == 0), stop=(hc == HC - 1))
                nc.tensor.matmul(vps2[:, 0:256], lhs, wv_sb[:, hc, 512:768],
                                 start=(hc == 0), stop=(hc == HC - 1))
            vst = ppool.tile([128, 769], BF16, tag="vst")
            nc.vector.tensor_add(out=vst[:, 0:512], in0=vps1, in1=bv_b[:, 0:512])
            nc.vector.tensor_add(out=vst[:, 512:768], in0=vps2[:, 0:256],
                                 in1=bv_b[:, 512:768])
            nc.vector.memset(vst[:, 768:769], 1.0)
            vd = vdram.tile([128, 769], BF16, tag="vd")
            nc.sync.dma_start(out=vd, in_=vst)
            v_tiles.append(vd)

    # Q projection (own 1024 columns, from xqT)
    for qb2 in range(NQ // 512):
        xt = xtp.tile([128, HC, 512], BF16, tag="xt")
        nc.sync.dma_start(
            out=xt, in_=xqT.ap().rearrange("(c p) k -> p c k", p=128)[:, :, qb2 * 512:(qb2 + 1) * 512])
        for oc in range(HC):
            qps = psum.tile([128, 512], F32, tag="acc512")
            for hc in range(HC):
                nc.tensor.matmul(qps, wq_sb[:, hc, oc * 128:(oc + 1) * 128],
                                 xt[:, hc, :], start=(hc == 0), stop=(hc == HC - 1))
            nc.vector.tensor_scalar_add(
                out=q_h[:, oc, qb2 * 512:(qb2 + 1) * 512], in0=qps,
                scalar1=bq_sb[:, oc:oc + 1])

    # Wo1/Wo2 reuse the weight pool slots (Wq/Wk/Wv are dead after phase B)
    wo1_sb = wpool.tile([128, HC, H], BF16, tag="w")
    wo2_sb = wpool.tile([128, HC, H], BF16, tag="w")
    nc.sync.dma_start(out=wo1_sb, in_=wo1T.ap().rearrange("(c p) o -> p c o", p=128))
    nc.sync.dma_start(out=wo2_sb, in_=wo2T.ap().rearrange("(c p) o -> p c o", p=128))

    # ---- phases C-F per query block ----
    for qb in range(NQ // QB):
        q0 = qb * QB
        # C: scores + exp + ctx accumulation over all key chunks
        cps1 = [psum.tile([128, 512], F32, tag="acc512", name=f"cps1_{qb}_{i}") for i in range(QB // 128)]
        cps2 = [psum.tile([128, 257], F32, tag="acc257", name=f"cps2_{qb}_{i}") for i in range(QB // 128)]
        for kc in range(KC):
            vt = vstr.tile([128, 769], BF16, tag="vt")
            nc.sync.dma_start(out=vt, in_=v_tiles[kc])
            sps = psum.tile([128, QB], F32, tag="sps")
            for hc in range(HC):
                nc.tensor.matmul(sps, k_h[:, hc, kc * 128:(kc + 1) * 128],
                                 q_h[:, hc, q0:q0 + QB],
                                 start=(hc == 0), stop=(hc == HC - 1))
            pt = ppool.tile([128, QB], BF16, tag="pt")
            nc.scalar.activation(out=pt, in_=sps,
                                 func=mybir.ActivationFunctionType.Exp,
                                 scale=msc_sb[:, kc:kc + 1])
            for qs in range(QB // 128):
                lhs = pt[:, qs * 128:(qs + 1) * 128]
                nc.tensor.matmul(cps1[qs], lhs, vt[:, 0:512],
                                 start=(kc == 0), stop=(kc == KC - 1))
                nc.tensor.matmul(cps2[qs], lhs, vt[:, 512:769],
                                 start=(kc == 0), stop=(kc == KC - 1))
        # normalize by rowsum (last ctx column) and cast to bf16
        ctx_ts = []
        for qs in range(QB // 128):
            rs = smallp.tile([128, 1], F32, tag="rs")
            nc.vector.reciprocal(rs, cps2[qs][:, 256:257])
            ctx_t = ctxp.tile([128, H], BF16, tag="ctx_t")
            nc.vector.tensor_scalar_mul(out=ctx_t[:, 0:512], in0=cps1[qs], scalar1=rs)
            nc.vector.tensor_scalar_mul(out=ctx_t[:, 512:768], in0=cps2[qs][:, 0:256],
                                        scalar1=rs)
            ctx_ts.append(ctx_t)
        # transpose ctx [q,h] -> ctx_h [h,q]
        ctx_h = ctxp.tile([128, HC, QB], BF16, tag="ctx_h")
        for qs in range(QB // 128):
            for hc in range(HC):
                tps = psum.tile([128, 128], BF16, tag="tps")
                nc.tensor.transpose(tps, ctx_ts[qs][:, hc * 128:(hc + 1) * 128], ident)
                nc.vector.tensor_copy(ctx_h[:, hc, qs * 128:(qs + 1) * 128], tps)

        # D-F per 128-token tile
        for qs in range(QB // 128):
            t0 = q0 + qs * 128

            def ln_block(src_h, slot, w_sb, xb, g_b, be_b, out_tile, out_dtype_bf):
                ops1 = psum.tile([128, 512], F32, tag="acc512")
                ops2 = psum.tile([128, 257], F32, tag="acc257")
                for hc in range(HC):
                    lhs = src_h[:, hc, slot * 128:(slot + 1) * 128]
                    nc.tensor.matmul(ops1, lhs, w_sb[:, hc, 0:512],
                                     start=(hc == 0), stop=(hc == HC - 1))
                    nc.tensor.matmul(ops2[:, 0:256], lhs, w_sb[:, hc, 512:768],
                                     start=(hc == 0), stop=(hc == HC - 1))
                xbt = resp.tile([128, H], F32, tag="xbt")
                nc.sync.dma_start(out=xbt, in_=xb.ap()[t0:t0 + 128, :])
                pre = h1p.tile([128, H], F32, tag="pre")
                nc.vector.tensor_add(out=pre[:, 0:512], in0=ops1, in1=xbt[:, 0:512])
                nc.vector.tensor_add(out=pre[:, 512:768], in0=ops2[:, 0:256],
                                     in1=xbt[:, 512:768])
                stats = smallp.tile([128, 3, 6], F32, tag="stats")
                for i in range(3):
                    nc.vector.bn_stats(out=stats[:, i, :], in_=pre[:, i * 256:(i + 1) * 256])
                mv = smallp.tile([128, 2], F32, tag="mv")
                nc.vector.bn_aggr(out=mv, in_=stats)
                sd = smallp.tile([128, 1], F32, tag="sd")
                nc.scalar.activation(out=sd, in_=mv[:, 1:2],
                                     func=mybir.ActivationFunctionType.Sqrt,
                                     bias=eps_sb)
                rstd = smallp.tile([128, 1], F32, tag="rstd")
                nc.vector.reciprocal(rstd, sd)
                nc.vector.tensor_scalar(out=pre, in0=pre, scalar1=mv[:, 0:1],
                                        scalar2=rstd, op0=mybir.AluOpType.subtract,
                                        op1=mybir.AluOpType.mult)
                tmp = h1p.tile([128, H], F32, tag="tmp")
                nc.vector.tensor_mul(out=tmp, in0=pre, in1=g_b)
                nc.vector.tensor_add(out=out_tile, in0=tmp, in1=be_b)

            h1_bf = h1p.tile([128, H], BF16, tag="h1bf")
            ln_block(ctx_h, qs, wo1_sb, xb1, g1_b, be1_b, h1_bf, True)

            h1_h = h1p.tile([128, HC, 128], BF16, tag="h1h")
            for hc in range(HC):
                tps = psum.tile([128, 128], BF16, tag="tps")
                nc.tensor.transpose(tps, h1_bf[:, hc * 128:(hc + 1) * 128], ident)
                nc.vector.tensor_copy(h1_h[:, hc, :], tps)

            o2 = outp.tile([128, H], F32, tag="o2")
            ln_block(h1_h, 0, wo2_sb, xb2, g2_b, be2_b, o2, False)
            nc.sync.dma_start(out=out.ap()[t0:t0 + 128, :], in_=o2)

    ctx.close()


_CACHE = {}


def _build():
    if "nc" in _CACHE:
        return _CACHE["nc"]
    nc = bacc.Bacc("TRN2", target_bir_lowering=False, debug=False,
                   enable_asserts=False, num_devices=NCORES)
    io = (
        nc.dram_tensor("xT", [H, S], BF16, kind="ExternalInput"),
        nc.dram_tensor("xqT", [H, NQ], BF16, kind="ExternalInput"),
        nc.dram_tensor("wqT", [H, H], BF16, kind="ExternalInput"),
        nc.dram_tensor("wkT", [H, H], BF16, kind="ExternalInput"),
        nc.dram_tensor("wvT", [H, H], BF16, kind="ExternalInput"),
        nc.dram_tensor("wo1T", [H, H], BF16, kind="ExternalInput"),
        nc.dram_tensor("wo2T", [H, H], BF16, kind="ExternalInput"),
        nc.dram_tensor("bq", [H], F32, kind="ExternalInput"),
        nc.dram_tensor("bk", [H], F32, kind="ExternalInput"),
        nc.dram_tensor("bv", [H], F32, kind="ExternalInput"),
        nc.dram_tensor("g1", [H], F32, kind="ExternalInput"),
        nc.dram_tensor("be1", [H], F32, kind="ExternalInput"),
        nc.dram_tensor("g2", [H], F32, kind="ExternalInput"),
        nc.dram_tensor("be2", [H], F32, kind="ExternalInput"),
        nc.dram_tensor("mscale", [S], F32, kind="ExternalInput"),
        nc.dram_tensor("xb1", [NQ, H], F32, kind="ExternalInput"),
        nc.dram_tensor("xb2", [NQ, H], F32, kind="ExternalInput"),
        nc.dram_tensor("out", [NQ, H], F32, kind="ExternalOutput"),
    )
    with tile.TileContext(nc) as tc:
        _emit(nc, tc, io)
    nc.compile()
    _CACHE["nc"] = nc
    return nc


def kernel(hidden_states, attention_mask, Wq, bq, Wk, bk, Wv, bv,
           Wo1, bo1, g1, beta1, Wo2, bo2, g2, beta2):
    from concourse.bass_utils import run_bass_kernel_spmd

    nc = _build()
    bf = ml_dtypes.bfloat16
    x = np.asarray(hidden_states, np.float32)
    mask = np.asarray(attention_mask, np.float32)

    shared = {
        "wqT": np.ascontiguousarray(np.asarray(Wq, np.float32).T).astype(bf),
        "wkT": np.ascontiguousarray(np.asarray(Wk, np.float32).T).astype(bf),
        "wvT": np.ascontiguousarray(np.asarray(Wv, np.float32).T).astype(bf),
        "wo1T": np.ascontiguousarray(np.asarray(Wo1, np.float32).T).astype(bf),
        "wo2T": np.ascontiguousarray(np.asarray(Wo2, np.float32).T).astype(bf),
        "bq": np.asarray(bq, np.float32), "bk": np.asarray(bk, np.float32),
        "bv": np.asarray(bv, np.float32),
        "g1": np.asarray(g1, np.float32), "be1": np.asarray(beta1, np.float32),
        "g2": np.asarray(g2, np.float32), "be2": np.asarray(beta2, np.float32),
    }
    in_maps = []
    for c in range(NCORES):
        b, qc = c // 4, c % 4
        xb = x[b]                                   # [S, H]
        xTb = np.ascontiguousarray(xb.T).astype(bf)  # [H, S]
        chunk = xb[qc * NQ:(qc + 1) * NQ]            # [NQ, H]
        m = {
            "xT": xTb,
            "xqT": np.ascontiguousarray(chunk.T).astype(bf),
            "mscale": (mask[b, 0] * np.float32(1.0 / np.sqrt(H))).astype(np.float32),
            "xb1": (chunk + np.asarray(bo1, np.float32)).astype(np.float32),
            "xb2": (chunk + np.asarray(bo2, np.float32)).astype(np.float32),
        }
        m.update(shared)
        in_maps.append(m)

    res = run_bass_kernel_spmd(nc, in_maps, core_ids=list(range(NCORES)))
    out = np.empty((B, S, H), np.float32)
    for c in range(NCORES):
        b, qc = c // 4, c % 4
        out[b, qc * NQ:(qc + 1) * NQ] = res.results[c]["out"]
    return out
